# revision 1
# baseline (speedup 1.0000x reference)
"""Trainium2 Bass kernel for nn_GAT_GCN (GAT -> GCN -> readout -> MLP), 8-core SPMD.

Sharding: data-parallel over graph-aligned node ranges. Cores 0-6 own 992
nodes (32 graphs x 31 nodes), core 7 owns 1056 (+64 orphan nodes that the
readout drops). Edges are owned by their dst node.

Algorithm (aggregate-then-transform, one-hot matmul aggregation):
- GAT: out_h = (A_h @ x) @ W_h. Per 128-dst block, bf16 one-hot masks
  M_h[e,d] = (dst_e==d) * exp(logit_eh) are built on-chip; aggregation and
  transform run on TensorE; the softmax 1/z normalization is folded into the
  per-partition `scale` operand of the ReLU activation that drains PSUM.
- h (bf16) is AllGathered; GCN gathers h rows per edge via indirect DMA.
- GCN: agg2.T = H_e.T @ M2 (norm-weighted one-hot), then h2.T = W_gcn.T @
  agg2.T per 128-feature chunk, fused with ReLU and the graph max/mean
  readout (31-node segments along the free dim). MLP runs transposed.
Each core outputs its 32 graphs as [64, 32]; the host concatenates.
"""
import sys
import numpy as np
import ml_dtypes

sys.path.insert(0, "/opt/trn_rl_repo")

from contextlib import ExitStack  # noqa: E402

import concourse.bass as bass  # noqa: E402
import concourse.tile as tile  # noqa: E402
from concourse import bacc, mybir  # noqa: E402

N, E, G = 8000, 32000, 256
F, H = 680, 10
HF = F * H                    # 6800
NC_ = 8                       # cores
NPC = 992                     # owned nodes per core (core 7: +64 orphans)
RPC = 1056                    # h_pad rows per core (padded slice size)
NBLK = 9                      # GAT dst blocks per core (128 each)
NBLK2 = 8                     # GCN dst blocks per core (992 readout rows)
EB = 768                      # padded edges per block
EC = EB // 128                # 6 edge chunks
FP = 768                      # padded F
KC1 = FP // 128               # 6
HFP = 6912                    # padded HF
KC2 = HFP // 128              # 54
HFT = HFP // 3                # 2304 (h stored/gathered in 3 column thirds)
GPC = 32                      # graphs per core
NPG = 31                      # nodes per graph

f32 = mybir.dt.float32
bf16 = mybir.dt.bfloat16
i32 = mybir.dt.int32
bfnp = ml_dtypes.bfloat16


# ----------------------------------------------------------------------------
# Host-side prep: sharding, padding, weight tiling
# ----------------------------------------------------------------------------

def host_prep(inputs):
    x = np.asarray(inputs["x"], np.float32)
    edge_index = np.asarray(inputs["edge_index"])
    W_gat = np.asarray(inputs["W_gat"], np.float32)
    a_src = np.asarray(inputs["a_src"], np.float32)
    a_dst = np.asarray(inputs["a_dst"], np.float32)
    W_gcn = np.asarray(inputs["W_gcn"], np.float32)
    W1 = np.asarray(inputs["W1"], np.float32)
    W2 = np.asarray(inputs["W2"], np.float32)
    W3 = np.asarray(inputs["W3"], np.float32)
    for bname in ("b_gat", "b_gcn", "b1", "b2", "b3"):
        assert np.all(np.asarray(inputs[bname]) == 0), f"nonzero {bname}"

    src = np.concatenate([edge_index[0], np.arange(N)]).astype(np.int64)
    dst = np.concatenate([edge_index[1], np.arange(N)]).astype(np.int64)
    deg = np.bincount(dst, minlength=N).astype(np.float64)
    dinv = 1.0 / np.sqrt(deg)
    norm = (dinv[src] * dinv[dst]).astype(np.float32)

    owner_n = np.minimum(np.arange(N) // NPC, NC_ - 1)
    hpos = RPC * owner_n + (np.arange(N) - NPC * owner_n)   # node -> hpad row

    xb = np.zeros((N, FP), bfnp)
    xb[:, :F] = x.astype(bfnp)

    As = np.stack([W_gat[:, h * F:(h + 1) * F] @ a_src[h] for h in range(H)], 1)
    Ad = np.stack([W_gat[:, h * F:(h + 1) * F] @ a_dst[h] for h in range(H)], 1)
    ascat = np.zeros((FP, 64), bfnp)
    ascat[:F, :H] = As.astype(bfnp)
    ascat[:F, H:2 * H] = Ad.astype(bfnp)

    wgat = np.zeros((KC1, 128, HF), bfnp)
    wgat.reshape(FP, HF)[:F] = W_gat.astype(bfnp)

    wpad = np.zeros((HFP, HFP), np.float32)
    wpad[:HF, :HF] = W_gcn
    # [fc', i(k row in chunk), kc, j] -> per-partition contiguous DMA slabs
    wgcn = np.ascontiguousarray(
        wpad.reshape(KC2, 128, KC2, 128).transpose(2, 1, 0, 3)).astype(bfnp)

    # gT k-order: 54 gmp chunks (rows [0,HF) + 16 pad) then 54 gap chunks
    w1t = np.zeros((2 * KC2, 128, 512), bfnp)
    w1t.reshape(2 * HFP, 512)[:HF] = W1[:HF].astype(bfnp)
    w1t.reshape(2 * HFP, 512)[HFP:HFP + HF] = W1[HF:].astype(bfnp)
    w2t = np.ascontiguousarray(W2.reshape(4, 128, 128)).astype(bfnp)
    w3t = np.ascontiguousarray(W3).astype(bfnp)              # [128, 64]

    shared = dict(xb=xb, ascat=ascat, wgat=wgat, wgcn=wgcn,
                  w1t=w1t, w2t=w2t, w3t=w3t)

    per_core = []
    for c in range(NC_):
        esrcx = np.zeros((NBLK, EC, 128), np.int32)
        esrch = np.zeros((NBLK, EC, 128), np.int32)
        edstg = np.zeros((NBLK, EC, 128), np.int32)
        dstrow = np.zeros((NBLK, 128), np.int32)
        dlocc = np.full((NBLK, EC, 128), -1.0, np.float32)
        normc = np.zeros((NBLK, EC, 128), np.float32)
        em = (np.minimum(dst // NPC, NC_ - 1) == c)
        es, ed, en = src[em], dst[em], norm[em]
        loc = ed - NPC * c
        for b in range(NBLK):
            bm = (loc >= 128 * b) & (loc < 128 * (b + 1))
            cnt = int(bm.sum())
            assert cnt <= EB, (c, b, cnt)
            fs = np.zeros(EB, np.int64)
            fd = np.zeros(EB, np.int64)
            fl = np.full(EB, -1.0, np.float32)
            fn = np.zeros(EB, np.float32)
            fs[:cnt] = es[bm]
            fd[:cnt] = ed[bm]
            fl[:cnt] = (loc[bm] - 128 * b).astype(np.float32)
            fn[:cnt] = en[bm]
            esrcx[b] = fs.reshape(EC, 128)
            esrch[b] = hpos[fs].reshape(EC, 128)
            edstg[b] = fd.reshape(EC, 128)
            dlocc[b] = fl.reshape(EC, 128)
            normc[b] = fn.reshape(EC, 128)
            dstrow[b] = np.clip(NPC * c + 128 * b + np.arange(128), 0, N - 1)
        pc = dict(
            esrcx=np.ascontiguousarray(esrcx.reshape(NBLK * EC, 128).T),
            esrch=np.ascontiguousarray(esrch[:NBLK2].reshape(NBLK2 * EC, 128).T),
            edstg=np.ascontiguousarray(edstg.reshape(NBLK * EC, 128).T),
            dlocc=np.ascontiguousarray(dlocc.reshape(NBLK * EC, 128).T),
            normc=np.ascontiguousarray(
                normc[:NBLK2].reshape(NBLK2 * EC, 128).T.astype(bfnp)),
        )
        per_core.append(pc)
    return shared, per_core


# ----------------------------------------------------------------------------
# Device program (one SPMD Bass program; all per-core variation is via data)
# ----------------------------------------------------------------------------

def build_nc():
    nc = bacc.Bacc("TRN2", target_bir_lowering=False, debug=False,
                   num_devices=NC_)
    xb = nc.dram_tensor("xb", [N, FP], bf16, kind="ExternalInput").ap()
    ascat = nc.dram_tensor("ascat", [FP, 64], bf16, kind="ExternalInput").ap()
    wgat = nc.dram_tensor("wgat", [KC1, 128, HF], bf16, kind="ExternalInput").ap()
    wgcn = nc.dram_tensor("wgcn", [KC2, 128, KC2, 128], bf16,
                          kind="ExternalInput").ap()
    w1t = nc.dram_tensor("w1t", [2 * KC2, 128, 512], bf16,
                         kind="ExternalInput").ap()
    w2t = nc.dram_tensor("w2t", [4, 128, 128], bf16, kind="ExternalInput").ap()
    w3t = nc.dram_tensor("w3t", [128, 64], bf16, kind="ExternalInput").ap()
    esrcx = nc.dram_tensor("esrcx", [128, NBLK * EC], i32,
                           kind="ExternalInput").ap()
    esrch = nc.dram_tensor("esrch", [128, NBLK2 * EC], i32,
                           kind="ExternalInput").ap()
    edstg = nc.dram_tensor("edstg", [128, NBLK * EC], i32,
                           kind="ExternalInput").ap()
    dlocc = nc.dram_tensor("dlocc", [128, NBLK * EC], f32,
                           kind="ExternalInput").ap()
    normc = nc.dram_tensor("normc", [128, NBLK2 * EC], bf16,
                           kind="ExternalInput").ap()
    outg = nc.dram_tensor("outg", [64, 32], f32, kind="ExternalOutput").ap()

    with tile.TileContext(nc) as tc, ExitStack() as ctx:
        dram = ctx.enter_context(tc.tile_pool(name="dram", bufs=1, space="DRAM"))
        alsp = dram.tile([N, 64], f32, name="alsp")
        hsls = [dram.tile([RPC, HFT], bf16, name=f"hsl{t}") for t in range(3)]
        hpads = [dram.tile([NC_ * RPC, HFT], bf16, name=f"hpad{t}",
                           addr_space="Shared") for t in range(3)]
        singles = ctx.enter_context(tc.tile_pool(name="singles", bufs=1))

        iota_i = singles.tile([128, 128], i32)
        nc.gpsimd.iota(iota_i, pattern=[[1, 128]], base=0, channel_multiplier=0)
        iota_f = singles.tile([128, 128], f32)
        nc.vector.tensor_copy(iota_f, iota_i)

        ascat_sb = singles.tile([128, KC1, 64], bf16)
        nc.sync.dma_start(out=ascat_sb,
                          in_=ascat.rearrange("(c p) d -> p c d", p=128))
        esrcx_sb = singles.tile([128, NBLK * EC], i32)
        nc.sync.dma_start(out=esrcx_sb, in_=esrcx)
        esrch_sb = singles.tile([128, NBLK2 * EC], i32)
        nc.sync.dma_start(out=esrch_sb, in_=esrch)
        edstg_sb = singles.tile([128, NBLK * EC], i32)
        nc.sync.dma_start(out=edstg_sb, in_=edstg)
        dlocc_sb = singles.tile([128, NBLK * EC], f32)
        nc.sync.dma_start(out=dlocc_sb, in_=dlocc)
        normc_sb = singles.tile([128, NBLK2 * EC], bf16)
        nc.sync.dma_start(out=normc_sb, in_=normc)

        # ---------------- Phase A: al = x @ [As|Ad] -> alsp ----------------
        from concourse.masks import make_identity
        ident = singles.tile([128, 128], bf16, name="ident")
        make_identity(nc, ident)
        with tc.tile_pool(name="pa_sb", bufs=3) as pool, \
             tc.tile_pool(name="pa_ps", bufs=2, space="PSUM") as pps, \
             tc.tile_pool(name="pa_pt", bufs=4, space="PSUM") as ppt:
            nchk = (N + 127) // 128
            for i in range(nchk):
                r0 = 128 * i
                nr = min(128, N - r0)
                xr = pool.tile([128, FP], bf16, tag="xr")
                nc.sync.dma_start(out=xr[:nr], in_=xb[r0:r0 + nr, :])
                if nr < 128:
                    nc.vector.memset(xr[nr:, :], 0.0)
                xt = pool.tile([128, KC1, 128], bf16, tag="xt")
                for k in range(KC1):
                    pt = ppt.tile([128, 128], bf16, tag="pt")
                    nc.tensor.transpose(
                        out=pt, in_=xr[:, 128 * k:128 * (k + 1)],
                        identity=ident)
                    nc.vector.tensor_copy(xt[:, k, :], pt)
                pal = pps.tile([128, 2 * H], f32, tag="pal")
                for k in range(KC1):
                    nc.tensor.matmul(pal[:nr], xt[:, k, :nr],
                                     ascat_sb[:, k, :2 * H],
                                     start=(k == 0), stop=(k == KC1 - 1))
                al_sb = pool.tile([128, 64], f32, tag="al")
                nc.vector.tensor_copy(al_sb[:nr, :2 * H], pal[:nr])
                nc.vector.memset(al_sb[:nr, 2 * H:], 0.0)
                nc.sync.dma_start(out=alsp[r0:r0 + nr, :], in_=al_sb[:nr])

        # ---------------- Phase B: GAT blocks -> hsl ----------------
        with tc.tile_pool(name="pb_w", bufs=1) as pw, \
             tc.tile_pool(name="pb_sb", bufs=2) as pool, \
             tc.tile_pool(name="pb_sm", bufs=3) as psm, \
             tc.tile_pool(name="pb_m", bufs=EC + 2) as pm, \
             tc.tile_pool(name="pb_ps", bufs=2, space="PSUM") as pps, \
             tc.tile_pool(name="pb_ph", bufs=1, space="PSUM") as pph, \
             tc.tile_pool(name="pb_pz", bufs=2, space="PSUM") as ppz:
            wgat_sb = pw.tile([128, KC1, HF], bf16)
            for k in range(KC1):
                nc.sync.dma_start(out=wgat_sb[:, k, :], in_=wgat[k])

            for b in range(NBLK):
                xe = pool.tile([128, EC, FP], bf16, tag="xe")
                als = pool.tile([128, EC, 64], f32, tag="als")
                ald = pool.tile([128, EC, 64], f32, tag="ald")
                for e in range(EC):
                    col = b * EC + e
                    nc.gpsimd.indirect_dma_start(
                        out=xe[:, e, :], out_offset=None, in_=xb,
                        in_offset=bass.IndirectOffsetOnAxis(
                            ap=esrcx_sb[:, col:col + 1], axis=0))
                    nc.gpsimd.indirect_dma_start(
                        out=als[:, e, :], out_offset=None, in_=alsp,
                        in_offset=bass.IndirectOffsetOnAxis(
                            ap=esrcx_sb[:, col:col + 1], axis=0))
                    nc.gpsimd.indirect_dma_start(
                        out=ald[:, e, :], out_offset=None, in_=alsp,
                        in_offset=bass.IndirectOffsetOnAxis(
                            ap=edstg_sb[:, col:col + 1], axis=0))

                masks = []
                exb = psm.tile([128, EC, H], bf16, tag="exb")
                for e in range(EC):
                    col = b * EC + e
                    msk = pm.tile([128, 128], bf16, tag="msk")
                    nc.vector.tensor_tensor(
                        out=msk,
                        in0=dlocc_sb[:, col:col + 1].to_broadcast([128, 128]),
                        in1=iota_f, op=mybir.AluOpType.is_equal)
                    masks.append(msk)
                    # logits -> exp (leaky_relu slope 0.2)
                    lg = psm.tile([128, H], f32, tag="lg")
                    nc.vector.tensor_tensor(out=lg, in0=als[:, e, :H],
                                            in1=ald[:, e, H:2 * H],
                                            op=mybir.AluOpType.add)
                    lg2 = psm.tile([128, H], f32, tag="lg2")
                    nc.vector.tensor_scalar_mul(lg2, lg, 0.2)
                    nc.vector.tensor_tensor(out=lg, in0=lg, in1=lg2,
                                            op=mybir.AluOpType.max)
                    nc.scalar.activation(out=exb[:, e, :], in_=lg,
                                         func=mybir.ActivationFunctionType.Exp)

                # z[d,h] = sum_e mask[e,d] * ex[e,h]
                pz = ppz.tile([128, H], f32, tag="pz")
                for e in range(EC):
                    nc.tensor.matmul(pz, masks[e], exb[:, e, :],
                                     start=(e == 0), stop=(e == EC - 1))
                zf = psm.tile([128, H], f32, tag="zf")
                nc.scalar.activation(out=zf, in_=pz,
                                     func=mybir.ActivationFunctionType.Copy,
                                     bias=1e-30)
                zinv = psm.tile([128, H], f32, tag="zinv")
                nc.vector.reciprocal(zinv, zf)

                # M_e[:, h, :] = mask_e * ex[e, h]
                aggT = pool.tile([128, KC1, H, 128], bf16, tag="aggT")
                Ms = []
                for e in range(EC):
                    Me = pm.tile([128, H, 128], bf16, tag="Me")
                    for h in range(H):
                        nc.vector.tensor_tensor(
                            out=Me[:, h, :], in0=masks[e],
                            in1=exb[:, e, h:h + 1].to_broadcast([128, 128]),
                            op=mybir.AluOpType.mult)
                    Ms.append(Me)
                # aggT[f, (h d)] += xe.T @ M
                for k in range(KC1):
                    for half in range(2):
                        pa = pps.tile([128, 5 * 128], f32, tag="pa")
                        h0 = 5 * half
                        for e in range(EC):
                            lhs = xe[:, e, 128 * k:128 * (k + 1)]
                            nc.tensor.matmul(pa[:, 0:512], lhs,
                                             Ms[e][:, h0:h0 + 4, :],
                                             start=(e == 0), stop=(e == EC - 1))
                            nc.tensor.matmul(pa[:, 512:640], lhs,
                                             Ms[e][:, h0 + 4:h0 + 5, :],
                                             start=(e == 0), stop=(e == EC - 1))
                        nc.vector.tensor_copy(aggT[:, k, h0:h0 + 5, :], pa)

                # transform per head + fused 1/z scale + relu
                h1 = pool.tile([128, HFP], bf16, tag="h1")
                for h in range(H):
                    ph = pph.tile([128, F], f32, tag="ph")
                    for k in range(KC1):
                        lhs = aggT[:, k, h, :]
                        nc.tensor.matmul(ph[:, 0:512], lhs,
                                         wgat_sb[:, k, F * h:F * h + 512],
                                         start=(k == 0), stop=(k == KC1 - 1))
                        nc.tensor.matmul(ph[:, 512:F], lhs,
                                         wgat_sb[:, k, F * h + 512:F * (h + 1)],
                                         start=(k == 0), stop=(k == KC1 - 1))
                    nc.scalar.activation(out=h1[:, F * h:F * (h + 1)], in_=ph,
                                         func=mybir.ActivationFunctionType.Relu,
                                         scale=zinv[:, h:h + 1])
                nc.vector.memset(h1[:, HF:HFP], 0.0)
                nr = min(128, RPC - 128 * b)
                for t in range(3):
                    nc.sync.dma_start(
                        out=hsls[t][128 * b:128 * b + nr, :],
                        in_=h1[:nr, HFT * t:HFT * (t + 1)])

        # ---------------- Phase C: AllGather h ----------------
        for t in range(3):
            nc.gpsimd.collective_compute(
                "AllGather", mybir.AluOpType.bypass,
                replica_groups=[list(range(NC_))],
                ins=[hsls[t]], outs=[hpads[t]])

        # ---------------- Phase D: GCN aggregation ----------------
        with tc.tile_pool(name="pd_a", bufs=1) as pagg:
          aggT2 = pagg.tile([128, KC2, 1024], bf16)
          with tc.tile_pool(name="pd_he", bufs=2 * EC) as phe, \
               tc.tile_pool(name="pd_m", bufs=EC + 2) as pm2, \
               tc.tile_pool(name="pd_ps", bufs=4, space="PSUM") as pps2:
            for b in range(NBLK2):
                M2s = []
                for e in range(EC):
                    col = b * EC + e
                    msk = pm2.tile([128, 128], bf16, tag="msk2")
                    nc.vector.tensor_tensor(
                        out=msk,
                        in0=dlocc_sb[:, col:col + 1].to_broadcast([128, 128]),
                        in1=iota_f, op=mybir.AluOpType.is_equal)
                    m2 = pm2.tile([128, 128], bf16, tag="m2")
                    nc.vector.tensor_tensor(
                        out=m2, in0=msk,
                        in1=normc_sb[:, col:col + 1].to_broadcast([128, 128]),
                        op=mybir.AluOpType.mult)
                    M2s.append(m2)
                for third in range(3):
                    hes = []
                    for e in range(EC):
                        col = b * EC + e
                        he = phe.tile([128, HFT], bf16, tag="he")
                        nc.gpsimd.indirect_dma_start(
                            out=he, out_offset=None, in_=hpads[third],
                            in_offset=bass.IndirectOffsetOnAxis(
                                ap=esrch_sb[:, col:col + 1], axis=0))
                        hes.append(he)
                    for kk in range(KC2 // 3):
                        kc = third * (KC2 // 3) + kk
                        p2 = pps2.tile([128, 128], f32, tag="p2")
                        for e in range(EC):
                            nc.tensor.matmul(
                                p2, hes[e][:, 128 * kk:128 * (kk + 1)], M2s[e],
                                start=(e == 0), stop=(e == EC - 1))
                        nc.vector.tensor_copy(
                            aggT2[:, kc, 128 * b:128 * (b + 1)], p2)

          # ---------------- Phase E: GCN transform + readout ----------------
          with tc.tile_pool(name="pe_g", bufs=1) as pg:
            gT = pg.tile([128, 2 * KC2, 32], bf16)
            with tc.tile_pool(name="pe_w", bufs=3) as pwp, \
                 tc.tile_pool(name="pe_sb", bufs=3) as pe, \
                 tc.tile_pool(name="pe_ps", bufs=2, space="PSUM") as pps3:
                for fp in range(KC2):
                    wsl = pwp.tile([128, KC2, 128], bf16, tag="wsl")
                    nc.sync.dma_start(out=wsl, in_=wgcn[fp])
                    ph2 = pps3.tile([128, NPC], f32, tag="ph2")
                    for kc in range(KC2):
                        nc.tensor.matmul(ph2[:, 0:512], wsl[:, kc, :],
                                         aggT2[:, kc, 0:512],
                                         start=(kc == 0), stop=(kc == KC2 - 1))
                        nc.tensor.matmul(ph2[:, 512:NPC], wsl[:, kc, :],
                                         aggT2[:, kc, 512:NPC],
                                         start=(kc == 0), stop=(kc == KC2 - 1))
                    h2c = pe.tile([128, NPC], bf16, tag="h2c")
                    nc.scalar.activation(out=h2c, in_=ph2,
                                         func=mybir.ActivationFunctionType.Relu)
                    h2r = h2c.rearrange("p (g n) -> p g n", n=NPG)
                    gmax = pe.tile([128, GPC], f32, tag="gmax")
                    nc.vector.tensor_reduce(out=gmax, in_=h2r,
                                            axis=mybir.AxisListType.X,
                                            op=mybir.AluOpType.max)
                    gsum = pe.tile([128, GPC], f32, tag="gsum")
                    nc.vector.tensor_reduce(out=gsum, in_=h2r,
                                            axis=mybir.AxisListType.X,
                                            op=mybir.AluOpType.add)
                    nc.vector.tensor_copy(gT[:, fp, :], gmax)
                    nc.scalar.activation(out=gT[:, KC2 + fp, :], in_=gsum,
                                         func=mybir.ActivationFunctionType.Copy,
                                         scale=1.0 / NPG)

            # ---------------- MLP (all transposed) ----------------
            with tc.tile_pool(name="pf_w", bufs=4) as pw1, \
                 tc.tile_pool(name="pf_sb", bufs=2) as pf, \
                 tc.tile_pool(name="pf_p1", bufs=4, space="PSUM") as pp1, \
                 tc.tile_pool(name="pf_p2", bufs=1, space="PSUM") as pp2:
                p1s = [pp1.tile([128, 32], f32, tag="p1", name=f"p1_{i}")
                       for i in range(4)]
                for kc in range(2 * KC2):
                    w1sl = pw1.tile([128, 512], bf16, tag="w1sl")
                    nc.sync.dma_start(out=w1sl, in_=w1t[kc])
                    for mc in range(4):
                        nc.tensor.matmul(
                            p1s[mc], w1sl[:, 128 * mc:128 * (mc + 1)],
                            gT[:, kc, :],
                            start=(kc == 0), stop=(kc == 2 * KC2 - 1))
                o1 = pf.tile([128, 4, 32], bf16, tag="o1")
                for mc in range(4):
                    nc.scalar.activation(
                        out=o1[:, mc, :], in_=p1s[mc],
                        func=mybir.ActivationFunctionType.Relu)
                w2sb = pf.tile([128, 4, 128], bf16, tag="w2sb")
                nc.sync.dma_start(out=w2sb,
                                  in_=w2t.rearrange("c p f -> p c f"))
                p2t = pp2.tile([128, 32], f32, tag="p2t")
                for kc in range(4):
                    nc.tensor.matmul(p2t, w2sb[:, kc, :], o1[:, kc, :],
                                     start=(kc == 0), stop=(kc == 3))
                o2 = pf.tile([128, 32], bf16, tag="o2")
                nc.vector.tensor_copy(o2, p2t)
                w3sb = pf.tile([128, 64], bf16, tag="w3sb")
                nc.sync.dma_start(out=w3sb, in_=w3t)
                p3t = pp2.tile([64, 32], f32, tag="p3t")
                nc.tensor.matmul(p3t, w3sb, o2, start=True, stop=True)
                o3 = pf.tile([64, 32], f32, tag="o3")
                nc.vector.tensor_copy(o3, p3t)
                nc.sync.dma_start(out=outg, in_=o3)

    nc.compile()
    return nc


_NC_CACHE = None


def get_nc():
    global _NC_CACHE
    if _NC_CACHE is None:
        _NC_CACHE = build_nc()
    return _NC_CACHE


def make_in_maps(inputs):
    shared, per_core = host_prep(inputs)
    return [dict(shared, **pc) for pc in per_core]


def kernel(**inputs):
    from concourse.bass_utils import run_bass_kernel_spmd
    nc = get_nc()
    in_maps = make_in_maps(inputs)
    res = run_bass_kernel_spmd(nc, in_maps, core_ids=list(range(NC_)))
    out = np.zeros((G, 64), np.float32)
    for c in range(NC_):
        out[GPC * c:GPC * (c + 1), :] = res.results[c]["outg"].T
    return out


if __name__ == "__main__":
    d = np.load("/root/problem/inputs.npz")
    inputs = {k: d[k] for k in d.files}
    out = kernel(**inputs)
    print("out", out.shape, out.dtype, out[:2, :4])

